# revision 32
# baseline (speedup 1.0000x reference)
"""Trainium2 Bass kernel for nn_Couple_loss_62380105007762.

Loss = w0 * MSE + w1 * KLD + w2 * CE where
  sig(x)  = 2 * x[:, 0].sum(axis=F)                      (inverse SSQ-STFT, real channel only)
  MSE     = sum((sig(output_rec) - sig(target_rec))**2)
  KLD     = -0.5 * sum(1 + log_var - mean**2 - exp(log_var))
  CE      = mean cross-entropy(output_clas, target_clas)

Sharding: data-parallel over the batch dim (64 rows -> 8 cores x 8 rows).
Each core computes a weighted partial loss scalar; host sums the 8 partials
(the "psum" of per-shard losses).

Device strategy per core (memory-bound; 2 x 8 MiB of f32 HBM traffic):
  - Stream the 8 [F=128, T=2048] real-channel planes of output_rec and
    target_rec on the two HWDGE rings (sync + scalar), bufs=8 so every DMA
    is issued up-front with no WAR back-pressure.
  - F-reduction via TensorE matmuls with ONE-HOT [128, 8] +/-1 weight
    columns: row b's +/-colsum lands on PSUM partition b. Four separate
    one-bank [8, 512] PSUM quarter tiles (a single [8, T] tile lets each
    quarter's Square read WAR-serialize the next quarter's matmuls).
  - ACT Square per quarter with accum_out -> per-row MSE; ACT lanes are
    partition-parallel so each costs ~0.7us, and the last row's matmuls go
    chunk-major so only the final quarter's Square trails the stream.
  - KLD/CE terms from a single packed [8, 526] side tensor; no row-max
    subtraction in CE (C=5, |logit| small -> exp can't overflow).
  - Per-row [mse, kld, ce, 1/8] columns reduced across partitions by a
    tiny ones-matmul, dotted with host-folded effective weights, DMA'd out.
"""

import numpy as np
from contextlib import ExitStack

import concourse.bass as bass
import concourse.tile as tile
from concourse import mybir
from concourse.bass_utils import run_bass_kernel_spmd
from concourse.compiler_utils import get_compiler_flags, set_compiler_flags


def _trim_dge_flags():
    """Drop unneeded DGE levels (spill_reload / io / scalar_dynamic_offset)
    from the compile flags: this kernel never spills and uses only static
    DMA offsets. Fewer enabled DGE queue sets measurably raises the
    sustained stream rate (~430 vs ~410 GB/s) — the SDMA engines round-robin
    fewer queue sets (io trim alone was worth ~8us end to end)."""
    flags = get_compiler_flags()
    out = []
    i = 0
    while i < len(flags):
        if flags[i] == "--internal-enable-dge-levels":
            out.append(flags[i])
            i += 1
            while i < len(flags) and not flags[i].startswith("-"):
                if flags[i] not in ("spill_reload", "io", "scalar_dynamic_offset"):
                    out.append(flags[i])
                i += 1
        else:
            out.append(flags[i])
            i += 1
    set_compiler_flags(out)

N_CORES = 8
B, Z, F, T, C = 64, 256, 128, 2048, 5
BS = B // N_CORES  # batch rows per core
N_CHUNK = 512      # PSUM bank = 512 f32 -> matmul out free dim per bank
SMW = 2 * Z + 2 * C + 7  # packed small tensor width: mean, lv, oc, onehot, w

FP32 = mybir.dt.float32
FP32R = mybir.dt.float32r
AX = mybir.AxisListType
ALU = mybir.AluOpType
ACTF = mybir.ActivationFunctionType


def build_bass(legalize: bool = True):
    nc = bass.Bass()

    # float32r: same 32-bit data, full-rate PE (1 cycle/row at free dim >= 256)
    o_rec = nc.declare_dram_parameter("o_rec", [BS, F, T], FP32R, isOutput=False)
    t_rec = nc.declare_dram_parameter("t_rec", [BS, F, T], FP32R, isOutput=False)
    small = nc.declare_dram_parameter("small", [BS, SMW], FP32, isOutput=False)
    # one-hot weight bank: col 7 = +1, col 22 = -1, else 0.
    # o-weights for row b = pm2[:, 7-b:15-b]  (+1 at relative col b)
    # t-weights for row b = pm2[:, 22-b:30-b] (-1 at relative col b)
    pm2 = nc.declare_dram_parameter("pm2", [F, 30], FP32R, isOutput=False)
    out = nc.declare_dram_parameter("out", [1, 1], FP32, isOutput=True)

    with tile.TileContext(nc) as tc:
        with ExitStack() as ctx:
            const_pool = ctx.enter_context(tc.tile_pool(name="const", bufs=1))
            o_pool = ctx.enter_context(tc.tile_pool(name="opool", bufs=BS))
            t_pool = ctx.enter_context(tc.tile_pool(name="tpool", bufs=BS))
            # PSUM: ps_all [8, T] f32 = 4 banks; ps_kc [1, 4] = 1 bank
            ps_pool = ctx.enter_context(tc.tile_pool(name="ps", bufs=1, space="PSUM"))
            psk_pool = ctx.enter_context(tc.tile_pool(name="psk", bufs=1, space="PSUM"))

            # pm2 rides near the head of the sync HWDGE ring (SWDGE packets
            # crawl ~9us for tiny strided tensors, and pm2 gates the first
            # matmul). It goes AFTER o0 so the big stream starts immediately;
            # pm2 still lands within ~0.1us of o0.
            pm2_t = const_pool.tile([F, 30], FP32R, tag="pm2")

            # ---- the plane stream. Two HWDGE queues only: a third (SWDGE)
            # queue lowers aggregate bandwidth (380 vs 410 GB/s measured),
            # and splitting planes shrinks descriptors / strands SDMA
            # engines (halves throughput, measured). Whole planes, rows
            # alternating rings (both rings measure ~equal, ~190-200 GB/s
            # effective each; asymmetric 9/7 splits measured worse).
            o_tiles, t_tiles = [], []
            HF = F // 2
            for b in range(BS):
                ot = o_pool.tile([F, T], FP32R, tag="o")
                tt = t_pool.tile([F, T], FP32R, tag="t")
                eng_o = nc.sync if b % 2 == 0 else nc.scalar
                eng_t = nc.scalar if b % 2 == 0 else nc.sync
                if b == BS - 1:
                    # Row 7 rides both rings as partition-halves (upper and
                    # lower 64 partitions hit complementary SDMA engine
                    # sets, descriptors stay 8 KB, rings stay 8/8
                    # balanced), so o7 completes ~one ring slot early and
                    # its matmuls overlap the remaining stream; only the
                    # t7 matmuls + final Square trail the last byte.
                    nc.sync.dma_start(ot[0:HF, :], o_rec[b, 0:HF, :])
                    nc.scalar.dma_start(ot[HF:F, :], o_rec[b, HF:F, :])
                    nc.sync.dma_start(tt[0:HF, :], t_rec[b, 0:HF, :])
                    nc.scalar.dma_start(tt[HF:F, :], t_rec[b, HF:F, :])
                else:
                    eng_o.dma_start(ot[:], o_rec[b, :, :])
                    eng_t.dma_start(tt[:], t_rec[b, :, :])
                o_tiles.append(ot)
                t_tiles.append(tt)
                if b == 0:
                    nc.sync.dma_start(pm2_t[:], pm2[:, :])

            # small feeds only slack-rich side terms; keep it OFF the ring
            # heads (a head slot delays every plane behind it by ~1.5us).
            sm = const_pool.tile([BS, SMW], FP32, tag="small")
            nc.scalar.dma_start(sm[:], small[:, :])

            # ---- small terms (KLD / CE rows) during the stream ----
            # kc[:, 0:4] = mse_row by column-quarter (filled by ACT accums)
            # kc[:, 4] = kld_row = sum(lv) - sum(m^2) - sum(exp(lv))
            # kc[:, 5] = ce_row  = log(sum(exp(oc))) - oc[y]
            # kc[:, 6] = 1/8     (carries the KLD "+1" constant via w_eff)
            kc = const_pool.tile([BS, 7], FP32, tag="kc")
            msq = const_pool.tile([BS, 1], FP32, tag="msq")
            esum = const_pool.tile([BS, 1], FP32, tag="esum")
            lvs = const_pool.tile([BS, 1], FP32, tag="lvs")
            sumexp = const_pool.tile([BS, 1], FP32, tag="sumexp")
            lse = const_pool.tile([BS, 1], FP32, tag="lse")
            picked = const_pool.tile([BS, 1], FP32, tag="picked")
            ktmp = const_pool.tile([BS, 1], FP32, tag="ktmp")
            junk_m = const_pool.tile([BS, Z], FP32, tag="jm")
            junk_lv = const_pool.tile([BS, Z], FP32, tag="jlv")
            junk_oc = const_pool.tile([BS, C], FP32, tag="joc")
            cej = const_pool.tile([BS, C], FP32, tag="cej")
            ones8 = const_pool.tile([BS, 1], FP32, tag="ones8")

            m_ap = sm[:, 0:Z]
            lv_ap = sm[:, Z:2 * Z]
            oc_ap = sm[:, 2 * Z:2 * Z + C]
            oh_ap = sm[:, 2 * Z + C:2 * Z + 2 * C]
            w_ap = sm[0:1, 2 * Z + 2 * C:2 * Z + 2 * C + 7]

            nc.scalar.activation(junk_m[:], m_ap, ACTF.Square, accum_out=msq[:])
            nc.scalar.activation(junk_lv[:], lv_ap, ACTF.Exp, accum_out=esum[:])
            nc.scalar.activation(junk_oc[:], oc_ap, ACTF.Exp, accum_out=sumexp[:])
            nc.scalar.activation(lse[:], sumexp[:], ACTF.Ln)
            nc.vector.reduce_sum(lvs[:], lv_ap, axis=AX.X)
            nc.vector.tensor_tensor(cej[:], oc_ap, oh_ap, ALU.mult)
            nc.vector.reduce_sum(picked[:], cej[:], axis=AX.X)
            nc.vector.tensor_tensor(ktmp[:], lvs[:], msq[:], ALU.subtract)
            nc.vector.tensor_tensor(kc[:, 4:5], ktmp[:], esum[:], ALU.subtract)
            nc.vector.tensor_tensor(kc[:, 5:6], lse[:], picked[:], ALU.subtract)
            nc.vector.memset(kc[:, 6:7], 1.0 / BS)
            nc.vector.memset(ones8[:], 1.0)

            # ---- main MSE stream: 64 accumulating matmuls ----
            # Four SEPARATE one-bank PSUM quarter tiles: with a single
            # [8, T] tile, each quarter's Square read creates a WAR edge
            # that serializes the next quarter's matmuls behind it
            # (measured +9us on the last row).
            nk = T // N_CHUNK
            ps_q = []
            for k in range(nk):
                psq = ps_pool.tile([BS, N_CHUNK], FP32, tag=f"ps{k}")
                ps_q.append(psq)
            big_junk = const_pool.tile([BS, T], FP32, tag="bjunk")
            for b in range(BS - 1):
                wo = pm2_t[:, 7 - b:15 - b]
                wt = pm2_t[:, 22 - b:30 - b]
                for k in range(nk):
                    sl = slice(k * N_CHUNK, (k + 1) * N_CHUNK)
                    nc.tensor.matmul(
                        ps_q[k][:], wo, o_tiles[b][:, sl],
                        start=(b == 0), stop=False, skip_group_check=True,
                    )
                for k in range(nk):
                    sl = slice(k * N_CHUNK, (k + 1) * N_CHUNK)
                    nc.tensor.matmul(
                        ps_q[k][:], wt, t_tiles[b][:, sl],
                        start=False, stop=False, skip_group_check=True,
                    )
            # Last row: all o-matmuls first (o7 completes before t7, so they
            # overlap the remaining stream), then t goes chunk-major with
            # each quarter's Square firing as soon as its accumulation
            # closes — only the final quarter's Square (~0.7us) trails the
            # last matmul instead of a full [8, T] pass (~2.2us).
            b = BS - 1
            wo = pm2_t[:, 7 - b:15 - b]
            wt = pm2_t[:, 22 - b:30 - b]
            for k in range(nk):
                sl = slice(k * N_CHUNK, (k + 1) * N_CHUNK)
                nc.tensor.matmul(
                    ps_q[k][:], wo, o_tiles[b][:, sl],
                    start=False, stop=False, skip_group_check=True,
                )
            for k in range(nk):
                sl = slice(k * N_CHUNK, (k + 1) * N_CHUNK)
                nc.tensor.matmul(
                    ps_q[k][:], wt, t_tiles[b][:, sl],
                    start=False, stop=True, skip_group_check=True,
                )
                nc.scalar.activation(
                    big_junk[:, sl], ps_q[k][:], ACTF.Square,
                    accum_out=kc[:, k:k + 1],
                )

            # partition-sum of kc[8, 7] -> psum [1, 7]; dot with w_eff
            ps_kc = psk_pool.tile([1, 7], FP32, tag="pskc")
            nc.tensor.matmul(ps_kc[:], ones8[:], kc[:], start=True, stop=True)
            vjunk = const_pool.tile([1, 7], FP32, tag="vjunk")
            res = const_pool.tile([1, 1], FP32, tag="res")
            nc.vector.tensor_tensor(vjunk[:], ps_kc[:], w_ap, ALU.mult)
            nc.vector.reduce_sum(res[:], vjunk[:], axis=AX.X)
            nc.sync.dma_start(out[:, :], res[:])

    # The SWDGE (Pool) dynamic queue carries no data DMAs in this kernel;
    # dropping its declaration removes 16 queue rings of NEFF exit-protocol
    # collateral (each engine re-arms every queue's semaphores serially at
    # the end of the body).
    nc.m.queues = [
        q for q in nc.m.queues if q.engine != mybir.EngineType.Pool
    ]

    if legalize:
        # CoreSim's race detector rejects the hoisted wait instructions
        # (no Tile fake sem updates), so sim runs build with legalize=False.
        _legalize_multi_waits(nc)
    # Populate .instr bytes for extended-ISA instructions — raw Bass skips
    # Bacc's lowering pass and the NEFF compiler fails without this.
    mybir.codegen_inst_isa_subclasses(nc)
    return nc


def _legalize_multi_waits(nc):
    """walrus rejects TPB compute instructions carrying more than one sync
    wait ("Too many sync wait commands" in the S3 encodings — hit for both
    Matmult/S3_LW and Activation/S3D3_AC). Hoist every wait of a multi-wait
    compute instruction onto standalone InstEventSemaphore instructions
    (exactly what `engine.wait_ge()` emits) inserted just before it on the
    same engine. DMA instructions keep their waits (DGE path handles many).
    """
    for fn in nc.m.functions:
        for blk in fn.blocks:
            new_insts = []
            for inst in blk.instructions:
                si = inst.sync_info
                tname = type(inst).__name__
                if (
                    si is not None
                    and si.on_wait
                    and len(si.on_wait) > 1
                    and tname != "InstEventSemaphore"
                ):
                    for i, w in enumerate(si.on_wait):
                        new_insts.append(
                            mybir.InstEventSemaphore(
                                name=f"{inst.name}_hoistw{i}",
                                engine=inst.engine,
                                ins=[],
                                outs=[],
                                sync_info=mybir.SyncInfo(on_wait=[w], on_update=[]),
                            )
                        )
                    inst.sync_info = mybir.SyncInfo(
                        on_wait=[], on_update=si.on_update
                    )
                new_insts.append(inst)
            blk.instructions = new_insts


_NC_CACHE = {}


def _get_nc():
    if "nc" not in _NC_CACHE:
        _trim_dge_flags()
        _NC_CACHE["nc"] = build_bass()
    return _NC_CACHE["nc"]


def make_in_maps(inputs) -> list[dict]:
    o = np.asarray(inputs["output_rec"], dtype=np.float32)
    t = np.asarray(inputs["target_rec"], dtype=np.float32)
    mean = np.asarray(inputs["mean"], dtype=np.float32)
    log_var = np.asarray(inputs["log_var"], dtype=np.float32)
    oclas = np.asarray(inputs["output_clas"], dtype=np.float32)
    tclas = np.asarray(inputs["target_clas"]).astype(np.int64)
    w = np.asarray(inputs["weight"], dtype=np.float32).astype(np.float64)

    # Only the real channel contributes to the inverse SSQ-STFT.
    o_real = np.ascontiguousarray(o[:, 0])  # [B, F, T]
    t_real = np.ascontiguousarray(t[:, 0])

    onehot = np.zeros((B, C), dtype=np.float32)
    onehot[np.arange(B), tclas] = 1.0

    # Effective weights folding ISSQ_SCALE^2=4 (MSE, one per column-quarter
    # accumulator), -0.5 (KLD), 1/B (CE mean) and the KLD sum-of-ones
    # constant (per-core 8*256 ones, carried by the kc[:, 6] = 1/8 column).
    w_eff = np.array(
        [4.0 * w[0]] * 4 + [-0.5 * w[1], w[2] / B, -0.5 * w[1] * (BS * Z)],
        dtype=np.float32,
    )

    small = np.zeros((B, SMW), dtype=np.float32)
    small[:, 0:Z] = mean
    small[:, Z:2 * Z] = log_var
    small[:, 2 * Z:2 * Z + C] = oclas
    small[:, 2 * Z + C:2 * Z + 2 * C] = onehot
    small[:, 2 * Z + 2 * C:] = w_eff[None, :]

    pm2 = np.zeros((F, 30), dtype=np.float32)
    pm2[:, 7] = 1.0
    pm2[:, 22] = -1.0

    in_maps = []
    for c in range(N_CORES):
        s = slice(c * BS, (c + 1) * BS)
        in_maps.append(
            {
                "o_rec": o_real[s],
                "t_rec": t_real[s],
                "small": small[s],
                "pm2": pm2,
            }
        )
    return in_maps


def kernel(**inputs) -> np.ndarray:
    in_maps = make_in_maps(inputs)
    nc = _get_nc()
    res = run_bass_kernel_spmd(nc, in_maps, list(range(N_CORES)))
    total = sum(float(r["out"][0, 0]) for r in res.results)
    return np.float32(total)


# revision 33
# speedup vs baseline: 1.3686x; 1.3686x over previous
"""Trainium2 Bass kernel for nn_Couple_loss_62380105007762.

Loss = w0 * MSE + w1 * KLD + w2 * CE where
  sig(x)  = 2 * x[:, 0].sum(axis=F)                      (inverse SSQ-STFT, real channel only)
  MSE     = sum((sig(output_rec) - sig(target_rec))**2)
  KLD     = -0.5 * sum(1 + log_var - mean**2 - exp(log_var))
  CE      = mean cross-entropy(output_clas, target_clas)

Sharding: data-parallel over the batch dim (64 rows -> 8 cores x 8 rows).
Each core computes a weighted partial loss scalar; host sums the 8 partials
(the "psum" of per-shard losses).

Device strategy per core (memory-bound; 2 x 8 MiB of f32 HBM traffic):
  - Stream the 8 [F=128, T=2048] real-channel planes of output_rec and
    target_rec on the two HWDGE rings (sync + scalar), bufs=8 so every DMA
    is issued up-front with no WAR back-pressure.
  - F-reduction via TensorE matmuls with ONE-HOT [128, 8] +/-1 weight
    columns: row b's +/-colsum lands on PSUM partition b. Four separate
    one-bank [8, 512] PSUM quarter tiles (a single [8, T] tile lets each
    quarter's Square read WAR-serialize the next quarter's matmuls).
  - ACT Square per quarter with accum_out -> per-row MSE; ACT lanes are
    partition-parallel so each costs ~0.7us, and the last row's matmuls go
    chunk-major so only the final quarter's Square trails the stream.
  - KLD/CE terms from a single packed [8, 526] side tensor; no row-max
    subtraction in CE (C=5, |logit| small -> exp can't overflow).
  - Per-row [mse, kld, ce, 1/8] columns reduced across partitions by a
    tiny ones-matmul, dotted with host-folded effective weights, DMA'd out.
"""

import numpy as np
from contextlib import ExitStack

import concourse.bass as bass
import concourse.tile as tile
from concourse import mybir
from concourse.bass_utils import run_bass_kernel_spmd
from concourse.compiler_utils import get_compiler_flags, set_compiler_flags


def _trim_dge_flags():
    """Drop unneeded DGE levels (spill_reload / io / scalar_dynamic_offset)
    from the compile flags: this kernel never spills and uses only static
    DMA offsets. Fewer enabled DGE queue sets measurably raises the
    sustained stream rate (~430 vs ~410 GB/s) — the SDMA engines round-robin
    fewer queue sets (io trim alone was worth ~8us end to end)."""
    flags = get_compiler_flags()
    out = []
    i = 0
    while i < len(flags):
        if flags[i] == "--internal-enable-dge-levels":
            out.append(flags[i])
            i += 1
            while i < len(flags) and not flags[i].startswith("-"):
                if flags[i] not in ("spill_reload", "io", "scalar_dynamic_offset"):
                    out.append(flags[i])
                i += 1
        else:
            out.append(flags[i])
            i += 1
    set_compiler_flags(out)

N_CORES = 8
B, Z, F, T, C = 64, 256, 128, 2048, 5
BS = B // N_CORES  # batch rows per core
N_CHUNK = 512      # PSUM bank = 512 f32 -> matmul out free dim per bank
SMW = 2 * Z + 2 * C + 7  # packed small tensor width: mean, lv, oc, onehot, w

FP32 = mybir.dt.float32
FP32R = mybir.dt.float32r
AX = mybir.AxisListType
ALU = mybir.AluOpType
ACTF = mybir.ActivationFunctionType


def build_bass(legalize: bool = True):
    nc = bass.Bass()

    # float32r: same 32-bit data, full-rate PE (1 cycle/row at free dim >= 256)
    o_rec = nc.declare_dram_parameter("o_rec", [BS, F, T], FP32R, isOutput=False)
    t_rec = nc.declare_dram_parameter("t_rec", [BS, F, T], FP32R, isOutput=False)
    small = nc.declare_dram_parameter("small", [BS, SMW], FP32, isOutput=False)
    # one-hot weight bank: col 7 = +1, col 22 = -1, else 0.
    # o-weights for row b = pm2[:, 7-b:15-b]  (+1 at relative col b)
    # t-weights for row b = pm2[:, 22-b:30-b] (-1 at relative col b)
    pm2 = nc.declare_dram_parameter("pm2", [F, 30], FP32R, isOutput=False)
    out = nc.declare_dram_parameter("out", [1, 1], FP32, isOutput=True)

    with tile.TileContext(nc) as tc:
        with ExitStack() as ctx:
            const_pool = ctx.enter_context(tc.tile_pool(name="const", bufs=1))
            o_pool = ctx.enter_context(tc.tile_pool(name="opool", bufs=BS))
            t_pool = ctx.enter_context(tc.tile_pool(name="tpool", bufs=BS))
            # PSUM: ps_all [8, T] f32 = 4 banks; ps_kc [1, 4] = 1 bank
            ps_pool = ctx.enter_context(tc.tile_pool(name="ps", bufs=1, space="PSUM"))
            psk_pool = ctx.enter_context(tc.tile_pool(name="psk", bufs=1, space="PSUM"))

            # pm2 rides near the head of the sync HWDGE ring (SWDGE packets
            # crawl ~9us for tiny strided tensors, and pm2 gates the first
            # matmul). It goes AFTER o0 so the big stream starts immediately;
            # pm2 still lands within ~0.1us of o0.
            pm2_t = const_pool.tile([F, 30], FP32R, tag="pm2")

            # ---- the plane stream. Two HWDGE queues only: a third (SWDGE)
            # queue lowers aggregate bandwidth (380 vs 410 GB/s measured),
            # and splitting planes shrinks descriptors / strands SDMA
            # engines (halves throughput, measured). Whole planes, rows
            # alternating rings (both rings measure ~equal, ~190-200 GB/s
            # effective each; asymmetric 9/7 splits measured worse).
            o_tiles, t_tiles = [], []
            for b in range(BS):
                ot = o_pool.tile([F, T], FP32R, tag="o")
                tt = t_pool.tile([F, T], FP32R, tag="t")
                eng_o = nc.sync if b % 2 == 0 else nc.scalar
                eng_t = nc.scalar if b % 2 == 0 else nc.sync
                eng_o.dma_start(ot[:], o_rec[b, :, :])
                eng_t.dma_start(tt[:], t_rec[b, :, :])
                o_tiles.append(ot)
                t_tiles.append(tt)
                if b == 0:
                    nc.sync.dma_start(pm2_t[:], pm2[:, :])

            # small feeds only slack-rich side terms; keep it OFF the ring
            # heads (a head slot delays every plane behind it by ~1.5us).
            sm = const_pool.tile([BS, SMW], FP32, tag="small")
            nc.scalar.dma_start(sm[:], small[:, :])

            # ---- small terms (KLD / CE rows) during the stream ----
            # kc[:, 0:4] = mse_row by column-quarter (filled by ACT accums)
            # kc[:, 4] = kld_row = sum(lv) - sum(m^2) - sum(exp(lv))
            # kc[:, 5] = ce_row  = log(sum(exp(oc))) - oc[y]
            # kc[:, 6] = 1/8     (carries the KLD "+1" constant via w_eff)
            kc = const_pool.tile([BS, 7], FP32, tag="kc")
            msq = const_pool.tile([BS, 1], FP32, tag="msq")
            esum = const_pool.tile([BS, 1], FP32, tag="esum")
            lvs = const_pool.tile([BS, 1], FP32, tag="lvs")
            sumexp = const_pool.tile([BS, 1], FP32, tag="sumexp")
            lse = const_pool.tile([BS, 1], FP32, tag="lse")
            picked = const_pool.tile([BS, 1], FP32, tag="picked")
            ktmp = const_pool.tile([BS, 1], FP32, tag="ktmp")
            junk_m = const_pool.tile([BS, Z], FP32, tag="jm")
            junk_lv = const_pool.tile([BS, Z], FP32, tag="jlv")
            junk_oc = const_pool.tile([BS, C], FP32, tag="joc")
            cej = const_pool.tile([BS, C], FP32, tag="cej")
            ones8 = const_pool.tile([BS, 1], FP32, tag="ones8")

            m_ap = sm[:, 0:Z]
            lv_ap = sm[:, Z:2 * Z]
            oc_ap = sm[:, 2 * Z:2 * Z + C]
            oh_ap = sm[:, 2 * Z + C:2 * Z + 2 * C]
            w_ap = sm[0:1, 2 * Z + 2 * C:2 * Z + 2 * C + 7]

            nc.scalar.activation(junk_m[:], m_ap, ACTF.Square, accum_out=msq[:])
            nc.scalar.activation(junk_lv[:], lv_ap, ACTF.Exp, accum_out=esum[:])
            nc.scalar.activation(junk_oc[:], oc_ap, ACTF.Exp, accum_out=sumexp[:])
            nc.scalar.activation(lse[:], sumexp[:], ACTF.Ln)
            nc.vector.reduce_sum(lvs[:], lv_ap, axis=AX.X)
            nc.vector.tensor_tensor(cej[:], oc_ap, oh_ap, ALU.mult)
            nc.vector.reduce_sum(picked[:], cej[:], axis=AX.X)
            nc.vector.tensor_tensor(ktmp[:], lvs[:], msq[:], ALU.subtract)
            nc.vector.tensor_tensor(kc[:, 4:5], ktmp[:], esum[:], ALU.subtract)
            nc.vector.tensor_tensor(kc[:, 5:6], lse[:], picked[:], ALU.subtract)
            nc.vector.memset(kc[:, 6:7], 1.0 / BS)
            nc.vector.memset(ones8[:], 1.0)

            # ---- main MSE stream: 64 accumulating matmuls ----
            # Four SEPARATE one-bank PSUM quarter tiles: with a single
            # [8, T] tile, each quarter's Square read creates a WAR edge
            # that serializes the next quarter's matmuls behind it
            # (measured +9us on the last row).
            nk = T // N_CHUNK
            ps_q = []
            for k in range(nk):
                psq = ps_pool.tile([BS, N_CHUNK], FP32, tag=f"ps{k}")
                ps_q.append(psq)
            big_junk = const_pool.tile([BS, T], FP32, tag="bjunk")
            for b in range(BS - 1):
                wo = pm2_t[:, 7 - b:15 - b]
                wt = pm2_t[:, 22 - b:30 - b]
                for k in range(nk):
                    sl = slice(k * N_CHUNK, (k + 1) * N_CHUNK)
                    nc.tensor.matmul(
                        ps_q[k][:], wo, o_tiles[b][:, sl],
                        start=(b == 0), stop=False, skip_group_check=True,
                    )
                for k in range(nk):
                    sl = slice(k * N_CHUNK, (k + 1) * N_CHUNK)
                    nc.tensor.matmul(
                        ps_q[k][:], wt, t_tiles[b][:, sl],
                        start=False, stop=False, skip_group_check=True,
                    )
            # Last row goes chunk-major, and each quarter's Square fires as
            # soon as that quarter's accumulation closes — only the final
            # quarter's Square (~0.7us) trails the last matmul instead of a
            # full [8, T] pass (~2.2us).
            b = BS - 1
            wo = pm2_t[:, 7 - b:15 - b]
            wt = pm2_t[:, 22 - b:30 - b]
            for k in range(nk):
                sl = slice(k * N_CHUNK, (k + 1) * N_CHUNK)
                nc.tensor.matmul(
                    ps_q[k][:], wo, o_tiles[b][:, sl],
                    start=False, stop=False, skip_group_check=True,
                )
                nc.tensor.matmul(
                    ps_q[k][:], wt, t_tiles[b][:, sl],
                    start=False, stop=True, skip_group_check=True,
                )
                nc.scalar.activation(
                    big_junk[:, sl], ps_q[k][:], ACTF.Square,
                    accum_out=kc[:, k:k + 1],
                )

            # partition-sum of kc[8, 7] -> psum [1, 7]; dot with w_eff
            ps_kc = psk_pool.tile([1, 7], FP32, tag="pskc")
            nc.tensor.matmul(ps_kc[:], ones8[:], kc[:], start=True, stop=True)
            vjunk = const_pool.tile([1, 7], FP32, tag="vjunk")
            res = const_pool.tile([1, 1], FP32, tag="res")
            nc.vector.tensor_tensor(vjunk[:], ps_kc[:], w_ap, ALU.mult)
            nc.vector.reduce_sum(res[:], vjunk[:], axis=AX.X)
            nc.sync.dma_start(out[:, :], res[:])

    # The SWDGE (Pool) dynamic queue carries no data DMAs in this kernel;
    # dropping its declaration removes 16 queue rings of NEFF exit-protocol
    # collateral (each engine re-arms every queue's semaphores serially at
    # the end of the body).
    nc.m.queues = [
        q for q in nc.m.queues if q.engine != mybir.EngineType.Pool
    ]

    if legalize:
        # CoreSim's race detector rejects the hoisted wait instructions
        # (no Tile fake sem updates), so sim runs build with legalize=False.
        _legalize_multi_waits(nc)
    # Populate .instr bytes for extended-ISA instructions — raw Bass skips
    # Bacc's lowering pass and the NEFF compiler fails without this.
    mybir.codegen_inst_isa_subclasses(nc)
    return nc


def _legalize_multi_waits(nc):
    """walrus rejects TPB compute instructions carrying more than one sync
    wait ("Too many sync wait commands" in the S3 encodings — hit for both
    Matmult/S3_LW and Activation/S3D3_AC). Hoist every wait of a multi-wait
    compute instruction onto standalone InstEventSemaphore instructions
    (exactly what `engine.wait_ge()` emits) inserted just before it on the
    same engine. DMA instructions keep their waits (DGE path handles many).
    """
    for fn in nc.m.functions:
        for blk in fn.blocks:
            new_insts = []
            for inst in blk.instructions:
                si = inst.sync_info
                tname = type(inst).__name__
                if (
                    si is not None
                    and si.on_wait
                    and len(si.on_wait) > 1
                    and tname != "InstEventSemaphore"
                ):
                    for i, w in enumerate(si.on_wait):
                        new_insts.append(
                            mybir.InstEventSemaphore(
                                name=f"{inst.name}_hoistw{i}",
                                engine=inst.engine,
                                ins=[],
                                outs=[],
                                sync_info=mybir.SyncInfo(on_wait=[w], on_update=[]),
                            )
                        )
                    inst.sync_info = mybir.SyncInfo(
                        on_wait=[], on_update=si.on_update
                    )
                new_insts.append(inst)
            blk.instructions = new_insts


_NC_CACHE = {}


def _get_nc():
    if "nc" not in _NC_CACHE:
        _trim_dge_flags()
        _NC_CACHE["nc"] = build_bass()
    return _NC_CACHE["nc"]


def make_in_maps(inputs) -> list[dict]:
    o = np.asarray(inputs["output_rec"], dtype=np.float32)
    t = np.asarray(inputs["target_rec"], dtype=np.float32)
    mean = np.asarray(inputs["mean"], dtype=np.float32)
    log_var = np.asarray(inputs["log_var"], dtype=np.float32)
    oclas = np.asarray(inputs["output_clas"], dtype=np.float32)
    tclas = np.asarray(inputs["target_clas"]).astype(np.int64)
    w = np.asarray(inputs["weight"], dtype=np.float32).astype(np.float64)

    # Only the real channel contributes to the inverse SSQ-STFT.
    o_real = np.ascontiguousarray(o[:, 0])  # [B, F, T]
    t_real = np.ascontiguousarray(t[:, 0])

    onehot = np.zeros((B, C), dtype=np.float32)
    onehot[np.arange(B), tclas] = 1.0

    # Effective weights folding ISSQ_SCALE^2=4 (MSE, one per column-quarter
    # accumulator), -0.5 (KLD), 1/B (CE mean) and the KLD sum-of-ones
    # constant (per-core 8*256 ones, carried by the kc[:, 6] = 1/8 column).
    w_eff = np.array(
        [4.0 * w[0]] * 4 + [-0.5 * w[1], w[2] / B, -0.5 * w[1] * (BS * Z)],
        dtype=np.float32,
    )

    small = np.zeros((B, SMW), dtype=np.float32)
    small[:, 0:Z] = mean
    small[:, Z:2 * Z] = log_var
    small[:, 2 * Z:2 * Z + C] = oclas
    small[:, 2 * Z + C:2 * Z + 2 * C] = onehot
    small[:, 2 * Z + 2 * C:] = w_eff[None, :]

    pm2 = np.zeros((F, 30), dtype=np.float32)
    pm2[:, 7] = 1.0
    pm2[:, 22] = -1.0

    in_maps = []
    for c in range(N_CORES):
        s = slice(c * BS, (c + 1) * BS)
        in_maps.append(
            {
                "o_rec": o_real[s],
                "t_rec": t_real[s],
                "small": small[s],
                "pm2": pm2,
            }
        )
    return in_maps


def kernel(**inputs) -> np.ndarray:
    in_maps = make_in_maps(inputs)
    nc = _get_nc()
    res = run_bass_kernel_spmd(nc, in_maps, list(range(N_CORES)))
    total = sum(float(r["out"][0, 0]) for r in res.results)
    return np.float32(total)


# revision 34
# speedup vs baseline: 1.3722x; 1.0026x over previous
"""Trainium2 Bass kernel for nn_Couple_loss_62380105007762.

Loss = w0 * MSE + w1 * KLD + w2 * CE where
  sig(x)  = 2 * x[:, 0].sum(axis=F)                      (inverse SSQ-STFT, real channel only)
  MSE     = sum((sig(output_rec) - sig(target_rec))**2)
  KLD     = -0.5 * sum(1 + log_var - mean**2 - exp(log_var))
  CE      = mean cross-entropy(output_clas, target_clas)

Sharding: data-parallel over the batch dim (64 rows -> 8 cores x 8 rows).
Each core computes a weighted partial loss scalar; host sums the 8 partials
(the "psum" of per-shard losses).

Device strategy per core (memory-bound; 2 x 8 MiB of f32 HBM traffic):
  - Stream the 8 [F=128, T=2048] real-channel planes of output_rec and
    target_rec on the two HWDGE rings (sync + scalar), bufs=8 so every DMA
    is issued up-front with no WAR back-pressure.
  - F-reduction via TensorE matmuls with ONE-HOT [128, 8] +/-1 weight
    columns: row b's +/-colsum lands on PSUM partition b. Four separate
    one-bank [8, 512] PSUM quarter tiles (a single [8, T] tile lets each
    quarter's Square read WAR-serialize the next quarter's matmuls).
  - ACT Square per quarter with accum_out -> per-row MSE; ACT lanes are
    partition-parallel so each costs ~0.7us, and the last row's matmuls go
    chunk-major so only the final quarter's Square trails the stream.
  - KLD/CE terms from a single packed [8, 526] side tensor; no row-max
    subtraction in CE (C=5, |logit| small -> exp can't overflow).
  - Per-row [mse, kld, ce, 1/8] columns reduced across partitions by a
    tiny ones-matmul, dotted with host-folded effective weights, DMA'd out.
"""

import numpy as np
from contextlib import ExitStack

import concourse.bass as bass
import concourse.tile as tile
from concourse import mybir
from concourse.bass_utils import run_bass_kernel_spmd
from concourse.compiler_utils import get_compiler_flags, set_compiler_flags


def _trim_dge_flags():
    """Drop unneeded DGE levels (spill_reload / io / scalar_dynamic_offset)
    from the compile flags: this kernel never spills and uses only static
    DMA offsets. Fewer enabled DGE queue sets measurably raises the
    sustained stream rate (~430 vs ~410 GB/s) — the SDMA engines round-robin
    fewer queue sets (io trim alone was worth ~8us end to end)."""
    flags = get_compiler_flags()
    out = []
    i = 0
    while i < len(flags):
        if flags[i] == "--internal-enable-dge-levels":
            out.append(flags[i])
            i += 1
            while i < len(flags) and not flags[i].startswith("-"):
                if flags[i] not in ("spill_reload", "io", "scalar_dynamic_offset"):
                    out.append(flags[i])
                i += 1
        else:
            out.append(flags[i])
            i += 1
    set_compiler_flags(out)

N_CORES = 8
B, Z, F, T, C = 64, 256, 128, 2048, 5
BS = B // N_CORES  # batch rows per core
N_CHUNK = 512      # PSUM bank = 512 f32 -> matmul out free dim per bank
SMW = 2 * Z + 2 * C + 7  # packed small tensor width: mean, lv, oc, onehot, w

FP32 = mybir.dt.float32
FP32R = mybir.dt.float32r
AX = mybir.AxisListType
ALU = mybir.AluOpType
ACTF = mybir.ActivationFunctionType


def build_bass(legalize: bool = True):
    nc = bass.Bass()

    # float32r: same 32-bit data, full-rate PE (1 cycle/row at free dim >= 256)
    o_rec = nc.declare_dram_parameter("o_rec", [BS, F, T], FP32R, isOutput=False)
    t_rec = nc.declare_dram_parameter("t_rec", [BS, F, T], FP32R, isOutput=False)
    small = nc.declare_dram_parameter("small", [BS, SMW], FP32, isOutput=False)
    # one-hot weight bank: col 7 = +1, col 22 = -1, else 0.
    # o-weights for row b = pm2[:, 7-b:15-b]  (+1 at relative col b)
    # t-weights for row b = pm2[:, 22-b:30-b] (-1 at relative col b)
    pm2 = nc.declare_dram_parameter("pm2", [F, 30], FP32R, isOutput=False)
    out = nc.declare_dram_parameter("out", [1, 1], FP32, isOutput=True)

    with tile.TileContext(nc) as tc:
        with ExitStack() as ctx:
            const_pool = ctx.enter_context(tc.tile_pool(name="const", bufs=1))
            o_pool = ctx.enter_context(tc.tile_pool(name="opool", bufs=BS))
            t_pool = ctx.enter_context(tc.tile_pool(name="tpool", bufs=BS))
            # PSUM: ps_all [8, T] f32 = 4 banks; ps_kc [1, 4] = 1 bank
            ps_pool = ctx.enter_context(tc.tile_pool(name="ps", bufs=1, space="PSUM"))
            psk_pool = ctx.enter_context(tc.tile_pool(name="psk", bufs=1, space="PSUM"))

            # pm2 rides near the head of the sync HWDGE ring (SWDGE packets
            # crawl ~9us for tiny strided tensors, and pm2 gates the first
            # matmul). It goes AFTER o0 so the big stream starts immediately;
            # pm2 still lands within ~0.1us of o0.
            pm2_t = const_pool.tile([F, 30], FP32R, tag="pm2")

            # ---- the plane stream. Two HWDGE queues only: a third (SWDGE)
            # queue lowers aggregate bandwidth (380 vs 410 GB/s measured),
            # and splitting planes shrinks descriptors / strands SDMA
            # engines (halves throughput, measured). Whole planes, rows
            # alternating rings (both rings measure ~equal, ~190-200 GB/s
            # effective each; asymmetric 9/7 splits measured worse).
            o_tiles, t_tiles = [], []
            for b in range(BS):
                ot = o_pool.tile([F, T], FP32R, tag="o")
                tt = t_pool.tile([F, T], FP32R, tag="t")
                eng_o = nc.sync if b % 2 == 0 else nc.scalar
                eng_t = nc.scalar if b % 2 == 0 else nc.sync
                eng_o.dma_start(ot[:], o_rec[b, :, :])
                eng_t.dma_start(tt[:], t_rec[b, :, :])
                o_tiles.append(ot)
                t_tiles.append(tt)
                if b == 0:
                    nc.sync.dma_start(pm2_t[:], pm2[:, :])

            # small feeds only slack-rich side terms; keep it OFF the ring
            # heads (a head slot delays every plane behind it by ~1.5us).
            sm = const_pool.tile([BS, SMW], FP32, tag="small")
            nc.scalar.dma_start(sm[:], small[:, :])

            # ---- small terms (KLD / CE rows) during the stream ----
            # kc[:, 0:4] = mse_row by column-quarter (filled by ACT accums)
            # kc[:, 4] = kld_row = sum(lv) - sum(m^2) - sum(exp(lv))
            # kc[:, 5] = ce_row  = log(sum(exp(oc))) - oc[y]
            # kc[:, 6] = 1/8     (carries the KLD "+1" constant via w_eff)
            kc = const_pool.tile([BS, 7], FP32, tag="kc")
            msq = const_pool.tile([BS, 1], FP32, tag="msq")
            esum = const_pool.tile([BS, 1], FP32, tag="esum")
            lvs = const_pool.tile([BS, 1], FP32, tag="lvs")
            sumexp = const_pool.tile([BS, 1], FP32, tag="sumexp")
            lse = const_pool.tile([BS, 1], FP32, tag="lse")
            picked = const_pool.tile([BS, 1], FP32, tag="picked")
            ktmp = const_pool.tile([BS, 1], FP32, tag="ktmp")
            junk_m = const_pool.tile([BS, Z], FP32, tag="jm")
            junk_lv = const_pool.tile([BS, Z], FP32, tag="jlv")
            junk_oc = const_pool.tile([BS, C], FP32, tag="joc")
            cej = const_pool.tile([BS, C], FP32, tag="cej")
            ones8 = const_pool.tile([BS, 1], FP32, tag="ones8")

            m_ap = sm[:, 0:Z]
            lv_ap = sm[:, Z:2 * Z]
            oc_ap = sm[:, 2 * Z:2 * Z + C]
            oh_ap = sm[:, 2 * Z + C:2 * Z + 2 * C]
            w_ap = sm[0:1, 2 * Z + 2 * C:2 * Z + 2 * C + 7]

            nc.scalar.activation(junk_m[:], m_ap, ACTF.Square, accum_out=msq[:])
            nc.scalar.activation(junk_lv[:], lv_ap, ACTF.Exp, accum_out=esum[:])
            nc.scalar.activation(junk_oc[:], oc_ap, ACTF.Exp, accum_out=sumexp[:])
            nc.scalar.activation(lse[:], sumexp[:], ACTF.Ln)
            nc.vector.reduce_sum(lvs[:], lv_ap, axis=AX.X)
            nc.vector.tensor_tensor(cej[:], oc_ap, oh_ap, ALU.mult)
            nc.vector.reduce_sum(picked[:], cej[:], axis=AX.X)
            nc.vector.tensor_tensor(ktmp[:], lvs[:], msq[:], ALU.subtract)
            nc.vector.tensor_tensor(kc[:, 4:5], ktmp[:], esum[:], ALU.subtract)
            nc.vector.tensor_tensor(kc[:, 5:6], lse[:], picked[:], ALU.subtract)
            nc.vector.memset(kc[:, 6:7], 1.0 / BS)
            nc.vector.memset(ones8[:], 1.0)

            # ---- main MSE stream: 64 accumulating matmuls ----
            # Four SEPARATE one-bank PSUM quarter tiles: with a single
            # [8, T] tile, each quarter's Square read creates a WAR edge
            # that serializes the next quarter's matmuls behind it
            # (measured +9us on the last row).
            nk = T // N_CHUNK
            ps_q = []
            for k in range(nk):
                psq = ps_pool.tile([BS, N_CHUNK], FP32, tag=f"ps{k}")
                ps_q.append(psq)
            big_junk = const_pool.tile([BS, T], FP32, tag="bjunk")
            for b in range(BS - 1):
                wo = pm2_t[:, 7 - b:15 - b]
                wt = pm2_t[:, 22 - b:30 - b]
                for k in range(nk):
                    sl = slice(k * N_CHUNK, (k + 1) * N_CHUNK)
                    nc.tensor.matmul(
                        ps_q[k][:], wo, o_tiles[b][:, sl],
                        start=(b == 0), stop=False, skip_group_check=True,
                    )
                for k in range(nk):
                    sl = slice(k * N_CHUNK, (k + 1) * N_CHUNK)
                    nc.tensor.matmul(
                        ps_q[k][:], wt, t_tiles[b][:, sl],
                        start=False, stop=False, skip_group_check=True,
                    )
            # Last row: the 4 o-matmuls run first (o7 sits one ring slot
            # before t7, so they overlap the t7 wait), then t goes
            # chunk-major with each quarter's Square firing as soon as its
            # accumulation closes — only the final quarter's Square
            # (~0.7us) trails the last matmul instead of a full [8, T]
            # pass (~2.2us).
            b = BS - 1
            wo = pm2_t[:, 7 - b:15 - b]
            wt = pm2_t[:, 22 - b:30 - b]
            for k in range(nk):
                sl = slice(k * N_CHUNK, (k + 1) * N_CHUNK)
                nc.tensor.matmul(
                    ps_q[k][:], wo, o_tiles[b][:, sl],
                    start=False, stop=False, skip_group_check=True,
                )
            for k in range(nk):
                sl = slice(k * N_CHUNK, (k + 1) * N_CHUNK)
                nc.tensor.matmul(
                    ps_q[k][:], wt, t_tiles[b][:, sl],
                    start=False, stop=True, skip_group_check=True,
                )
                nc.scalar.activation(
                    big_junk[:, sl], ps_q[k][:], ACTF.Square,
                    accum_out=kc[:, k:k + 1],
                )

            # partition-sum of kc[8, 7] -> psum [1, 7]; dot with w_eff
            ps_kc = psk_pool.tile([1, 7], FP32, tag="pskc")
            nc.tensor.matmul(ps_kc[:], ones8[:], kc[:], start=True, stop=True)
            vjunk = const_pool.tile([1, 7], FP32, tag="vjunk")
            res = const_pool.tile([1, 1], FP32, tag="res")
            nc.vector.tensor_tensor(vjunk[:], ps_kc[:], w_ap, ALU.mult)
            nc.vector.reduce_sum(res[:], vjunk[:], axis=AX.X)
            nc.sync.dma_start(out[:, :], res[:])

    # The SWDGE (Pool) dynamic queue carries no data DMAs in this kernel;
    # dropping its declaration removes 16 queue rings of NEFF exit-protocol
    # collateral (each engine re-arms every queue's semaphores serially at
    # the end of the body).
    nc.m.queues = [
        q for q in nc.m.queues if q.engine != mybir.EngineType.Pool
    ]

    if legalize:
        # CoreSim's race detector rejects the hoisted wait instructions
        # (no Tile fake sem updates), so sim runs build with legalize=False.
        _legalize_multi_waits(nc)
    # Populate .instr bytes for extended-ISA instructions — raw Bass skips
    # Bacc's lowering pass and the NEFF compiler fails without this.
    mybir.codegen_inst_isa_subclasses(nc)
    return nc


def _legalize_multi_waits(nc):
    """walrus rejects TPB compute instructions carrying more than one sync
    wait ("Too many sync wait commands" in the S3 encodings — hit for both
    Matmult/S3_LW and Activation/S3D3_AC). Hoist every wait of a multi-wait
    compute instruction onto standalone InstEventSemaphore instructions
    (exactly what `engine.wait_ge()` emits) inserted just before it on the
    same engine. DMA instructions keep their waits (DGE path handles many).
    """
    for fn in nc.m.functions:
        for blk in fn.blocks:
            new_insts = []
            for inst in blk.instructions:
                si = inst.sync_info
                tname = type(inst).__name__
                if (
                    si is not None
                    and si.on_wait
                    and len(si.on_wait) > 1
                    and tname != "InstEventSemaphore"
                ):
                    for i, w in enumerate(si.on_wait):
                        new_insts.append(
                            mybir.InstEventSemaphore(
                                name=f"{inst.name}_hoistw{i}",
                                engine=inst.engine,
                                ins=[],
                                outs=[],
                                sync_info=mybir.SyncInfo(on_wait=[w], on_update=[]),
                            )
                        )
                    inst.sync_info = mybir.SyncInfo(
                        on_wait=[], on_update=si.on_update
                    )
                new_insts.append(inst)
            blk.instructions = new_insts


_NC_CACHE = {}


def _get_nc():
    if "nc" not in _NC_CACHE:
        _trim_dge_flags()
        _NC_CACHE["nc"] = build_bass()
    return _NC_CACHE["nc"]


def make_in_maps(inputs) -> list[dict]:
    o = np.asarray(inputs["output_rec"], dtype=np.float32)
    t = np.asarray(inputs["target_rec"], dtype=np.float32)
    mean = np.asarray(inputs["mean"], dtype=np.float32)
    log_var = np.asarray(inputs["log_var"], dtype=np.float32)
    oclas = np.asarray(inputs["output_clas"], dtype=np.float32)
    tclas = np.asarray(inputs["target_clas"]).astype(np.int64)
    w = np.asarray(inputs["weight"], dtype=np.float32).astype(np.float64)

    # Only the real channel contributes to the inverse SSQ-STFT.
    o_real = np.ascontiguousarray(o[:, 0])  # [B, F, T]
    t_real = np.ascontiguousarray(t[:, 0])

    onehot = np.zeros((B, C), dtype=np.float32)
    onehot[np.arange(B), tclas] = 1.0

    # Effective weights folding ISSQ_SCALE^2=4 (MSE, one per column-quarter
    # accumulator), -0.5 (KLD), 1/B (CE mean) and the KLD sum-of-ones
    # constant (per-core 8*256 ones, carried by the kc[:, 6] = 1/8 column).
    w_eff = np.array(
        [4.0 * w[0]] * 4 + [-0.5 * w[1], w[2] / B, -0.5 * w[1] * (BS * Z)],
        dtype=np.float32,
    )

    small = np.zeros((B, SMW), dtype=np.float32)
    small[:, 0:Z] = mean
    small[:, Z:2 * Z] = log_var
    small[:, 2 * Z:2 * Z + C] = oclas
    small[:, 2 * Z + C:2 * Z + 2 * C] = onehot
    small[:, 2 * Z + 2 * C:] = w_eff[None, :]

    pm2 = np.zeros((F, 30), dtype=np.float32)
    pm2[:, 7] = 1.0
    pm2[:, 22] = -1.0

    in_maps = []
    for c in range(N_CORES):
        s = slice(c * BS, (c + 1) * BS)
        in_maps.append(
            {
                "o_rec": o_real[s],
                "t_rec": t_real[s],
                "small": small[s],
                "pm2": pm2,
            }
        )
    return in_maps


def kernel(**inputs) -> np.ndarray:
    in_maps = make_in_maps(inputs)
    nc = _get_nc()
    res = run_bass_kernel_spmd(nc, in_maps, list(range(N_CORES)))
    total = sum(float(r["out"][0, 0]) for r in res.results)
    return np.float32(total)
